# revision 29
# baseline (speedup 1.0000x reference)
"""Trainium2 Bass kernel v4 for NBFNet: exact backward-dependency-cone
truncation (score only needs hidden[t_idx]; restrict each layer to
FW_{l+1} & N_{l+1} nodes), host-side generic evolution for untouched
nodes, compact per-layer AllGather tables, single-strip DVE trees."""
import os
import sys
import types
import numpy as np

N = 50000
R = 100
D = 32
L = 6
B = 4
NQ = 4
EPS = 1e-6
NCORES = 8
NPC0 = N // NCORES
NSEEDN = 4
ZERO_ENTRY = 2 * R          # rel palette: [0,2R) rels, 2R zero, 2R+1.. bnd
ZERO_ROW = 0
ONES_ROW = 1
PAT_BASE = 2
SROWS = 16
SLABTAIL = 1536
MERGE_SLOTS = 96
BUCKETS = np.array([1, 2, 3, 4, 6, 8, 10, 12, 16, 20, 24, 32, 48, 64, 96,
                    128, 192, 256, 384, 512], dtype=np.int64)


def _env_setup():
    if "/opt/trn_rl_repo" not in sys.path:
        sys.path.insert(0, "/opt/trn_rl_repo")
    try:
        import antenv  # noqa
        if "antenv.axon_hooks" not in sys.modules:
            hook = [None]
            mod = types.ModuleType("antenv.axon_hooks")
            mod.set_axon_ntff_profile_hook = lambda h: hook.__setitem__(0, h)
            mod.get_axon_ntff_profile_hook = lambda: hook[0]
            sys.modules["antenv.axon_hooks"] = mod
            antenv.axon_hooks = mod
            try:
                sys.path.insert(0, "/root/.axon_site/trn_agent_boot")
                import trn_boot
                mod.set_axon_ntff_profile_hook(
                    trn_boot._ntff_profile_via_ctypes("/opt/axon/libaxon_pjrt.so"))
            except Exception:
                pass
    except Exception:
        pass


_env_setup()

import ml_dtypes  # noqa: E402

bf16 = ml_dtypes.bfloat16


def _bucket(x):
    return BUCKETS[np.searchsorted(BUCKETS, x)]


def _rup(x, m):
    return (int(x) + m - 1) // m * m


def _wrap_idx(v):
    n = len(v)
    assert n % 16 == 0
    a = np.asarray(v, dtype=np.int16).reshape(n // 16, 16).T
    return np.tile(a, (8, 1))


def build_host(inputs):
    el = np.asarray(inputs["edge_list"])
    src = el[:, 0].astype(np.int64)
    dst = el[:, 1].astype(np.int64)
    rel = el[:, 2].astype(np.int64)
    h_index = np.asarray(inputs["h_index"])
    r_index = np.asarray(inputs["r_index"])
    t_index = np.asarray(inputs["t_index"])
    query_emb = np.asarray(inputs["query_emb"], np.float64)
    lin_W = np.asarray(inputs["lin_W"], np.float64)
    lin_b = np.asarray(inputs["lin_b"], np.float64)
    h0 = h_index[:, 0].astype(np.int64)
    r0 = r_index[:, 0].astype(np.int64)
    query = query_emb[r0]

    # forward wavefront (value-based, per-query then union, as reference)
    T = np.zeros((B, N), dtype=bool)
    T[np.arange(B), h0] = True
    FW = []
    for l in range(L + 1):
        FW.append(T.any(0).copy())
        if l < L:
            for q in range(B):
                T[q, dst[T[q, src]]] = True

    # backward needed sets
    tgts = np.unique(t_index)
    Nl = [None] * (L + 1)
    m = np.zeros(N, dtype=bool)
    m[tgts] = True
    Nl[L] = m
    for l in range(L - 1, -1, -1):
        m2 = Nl[l + 1].copy()
        m2[src[Nl[l + 1][dst]]] = True
        Nl[l] = m2
    US = [FW[l + 1] & Nl[l + 1] for l in range(L)]

    # per-node constants (exact, host fp64)
    indeg = np.bincount(dst, minlength=N)
    degree = indeg.astype(np.float64) + 1.0
    scale = np.log(degree)
    scale = scale / scale.mean()
    iscale = 1.0 / np.clip(scale, 1e-2, None)
    rcnt = 1.0 / degree

    # seeds + boundary patterns
    seeds = np.unique(h0)
    pat = {}
    for n in seeds:
        p = np.zeros(NQ * D)
        for q in range(B):
            if h0[q] == n:
                p[q * D:(q + 1) * D] += query[q]
        pat[int(n)] = p
    is_seed = np.zeros(N, dtype=bool)
    is_seed[seeds] = True
    bnd_entry_of = {int(n): ZERO_ENTRY + 1 + j for j, n in enumerate(seeds)}
    pat_row_of = {int(n): PAT_BASE + j for j, n in enumerate(seeds)}
    NRELE = ZERO_ENTRY + 1 + len(seeds)

    # generic (untouched-node) evolution, host fp64, only for needed nodes
    need_g = np.zeros(N, dtype=bool)
    for l in range(1, L):
        need_g |= US[l] & ~US[l - 1]
    need_g |= Nl[L] & ~US[L - 1]
    need_g &= ~is_seed
    gnodes = np.nonzero(need_g)[0]
    gidx_of = np.full(N, -1, dtype=np.int64)
    gidx_of[gnodes] = np.arange(len(gnodes))
    gsnap = [np.zeros((len(gnodes), B, D))]
    gcur = gsnap[0]
    seps = np.sqrt(EPS)
    for l in range(L):
        Wh = lin_W[l][:D]
        Wu = lin_W[l][D:].reshape(D, 4, 3, D)
        Av = Wu[:, 3, 0, :].sum(0)
        Bv = Wu[:, 3, 1, :].sum(0)
        Cv = Wu[:, 3, 2, :].sum(0)
        std_term = seps * (Av[None, :] + scale[gnodes, None] * Bv[None, :]
                           + iscale[gnodes, None] * Cv[None, :])
        gcur = np.maximum(
            gcur @ Wh + std_term[:, None, :] + lin_b[l][None, None, :], 0.0)
        gsnap.append(gcur)

    # pre-pass: gext sizes (per-core new non-seed cols for layers >= 1)
    core_of = lambda n: n // NPC0  # noqa: E731
    gext_count = np.zeros(NCORES, dtype=np.int64)
    for l in range(1, L):
        new = US[l] & ~US[l - 1] & ~is_seed
        if int(US[l].sum()) + NSEEDN <= 508:
            gext_count += int(new.sum())
        else:
            for n in np.nonzero(new)[0]:
                gext_count[core_of(n)] += 1
    GE = _rup(gext_count.max(), 16) if gext_count.max() else 16
    ag0 = SROWS

    e_by_dst = np.argsort(dst, kind="stable")
    dst_s = dst[e_by_dst]

    prev_col = np.full(N, -1, dtype=np.int64)
    prev_Cp = 0
    prev_colrow0 = None
    prev_colmul = None
    agoff = [0] * (L + 1)   # agoff[l] = table offset of region used at layer l
    nxt = ag0

    layers = []
    percore = [dict(idxh=[], relc=[], idxp=[], helpers=[], corr=[],
                    gext_rows=[], colmap=[]) for _ in range(NCORES)]

    for l in range(L):
        e_act = FW[l][src] & US[l][dst]
        k_glob = np.bincount(dst[e_act], minlength=N)

        repl = (int(US[l].sum()) + NSEEDN <= 508) and l < L - 1
        if repl:
            rng = np.arange(N)
            usn = rng[US[l]]
            core_stat = [dict(n0=0, npc=N, usn=usn, sd=usn[is_seed[usn]])]
        else:
            core_stat = []
            for c in range(NCORES):
                n0 = c * NPC0
                rng = np.arange(n0, n0 + NPC0)
                usn = rng[US[l][rng]]
                sd = usn[is_seed[usn]]
                core_stat.append(dict(n0=n0, npc=NPC0, usn=usn, sd=sd))

        wseed = 2
        for st in core_stat:
            if len(st["sd"]):
                wseed = max(wseed, int(_bucket(k_glob[st["sd"]].max() + 1)))

        ncl = len(BUCKETS)
        Cmat = np.zeros((len(core_stat), ncl), dtype=np.int64)
        Zv = np.zeros(len(core_stat), dtype=np.int64)
        for c, st in enumerate(core_stat):
            nonseed = st["usn"][~is_seed[st["usn"]]]
            act = nonseed[k_glob[nonseed] > 0]
            st["act"] = act
            st["zk"] = nonseed[k_glob[nonseed] == 0]
            st["wb"] = np.searchsorted(BUCKETS, k_glob[act])
            Cmat[c] = np.bincount(st["wb"], minlength=ncl)
            Zv[c] = len(st["zk"])

        suf = Cmat[:, ::-1].cumsum(1)[:, ::-1]
        M = suf.max(0)
        counts = M - np.append(M[1:], 0)
        # merge small classes upward (fewer DVE tree ops; bounded slot pad)
        while True:
            nz = [i for i in range(ncl) if counts[i] > 0]
            if len(nz) < 2:
                break
            best = None
            for a, b2 in zip(nz, nz[1:]):
                cost = counts[a] * (BUCKETS[b2] - BUCKETS[a])
                if best is None or cost < best[0]:
                    best = (cost, a, b2)
            if best[0] > MERGE_SLOTS:
                break
            counts[best[2]] += counts[best[1]]
            counts[best[1]] = 0
        Z = int(Zv.max())
        cls_idx = [i for i in range(ncl) if counts[i] > 0]
        cls = [(int(BUCKETS[i]), int(counts[i])) for i in cls_idx]
        A = NSEEDN + int(counts.sum())
        C = A + Z
        Cp = _rup(max(C, 1), 128)
        assert Cp <= SLABTAIL

        w_seq = np.zeros(A, dtype=np.int64)
        w_seq[:NSEEDN] = wseed
        o = NSEEDN
        for w, nn_ in cls:
            w_seq[o:o + nn_] = w
            o += nn_
        off = np.zeros(A + 1, dtype=np.int64)
        np.cumsum(w_seq, out=off[1:])
        S = int(off[-1])

        runs = [(0, 0, NSEEDN, wseed)]
        soff, noff = NSEEDN * wseed, NSEEDN
        for w, nn_ in cls:
            runs.append((soff, noff, nn_, w))
            soff += nn_ * w
            noff += nn_

        # chunk the column space; small first chunk hides gather latency
        bounds = [0]
        rem = Cp
        if rem > 128:
            bounds.append(128)
            rem -= 128
        while rem > 0:
            step = min(512, rem)
            bounds.append(bounds[-1] + step)
            rem -= step
        chunks = []
        sbase = 0
        rowbase = 0
        rank_base = np.full(A, -1, dtype=np.int64)
        rank_stride = np.zeros(A, dtype=np.int64)
        colrow0 = np.zeros(Cp, dtype=np.int64)
        colmul = np.zeros(Cp, dtype=np.int64)
        for c0, c1 in zip(bounds[:-1], bounds[1:]):
            cruns = []
            so = 0
            for (rtoff, rtnode, rnseg, rw) in runs:
                a2 = max(rtnode, c0)
                b2 = min(rtnode + rnseg, c1)
                if a2 >= b2:
                    continue
                ns2 = b2 - a2
                cruns.append((so, a2 - c0, ns2, rw))
                rank_base[a2:b2] = sbase + so + np.arange(ns2)
                rank_stride[a2:b2] = ns2
                so += ns2 * rw
            Sk = so
            Xpk = _rup(max(Sk, 16), 128)
            colrow0[c0:c1] = rowbase + np.arange(c1 - c0)
            colmul[c0:c1] = c1 - c0
            chunks.append(dict(c0=c0, ncols=c1 - c0, runs=cruns, S=Sk,
                               Xp=Xpk, sbase=sbase, rowbase=rowbase))
            sbase += Xpk
            rowbase += NCORES * (c1 - c0)
        Xp = sbase

        layers.append(dict(wseed=wseed, cls=cls, A=A, C=C, Cp=Cp, S=S,
                           Xp=Xp, runs=runs, chunks=chunks, repl=repl,
                           agoff=agoff[l]))

        if repl:
            colrow0 = np.arange(Cp, dtype=np.int64)
            colmul = np.zeros(Cp, dtype=np.int64)
        src_is_slab = (l == 0) or layers[l - 1]["repl"] if l > 0 else True
        ones_pad = (SLABTAIL + ONES_ROW) if src_is_slab else ONES_ROW
        new_col = np.full(N, -1, dtype=np.int64)
        for c in range(NCORES):
            st = core_stat[0 if repl else c]
            n0 = st["n0"]
            npc = st["npc"]
            # non-US nodes usable as dummies
            rngl = np.arange(n0, n0 + npc)
            dummies = rngl[~US[l][rngl]][::-1]
            di = 0

            region = list(st["sd"])
            while len(region) < NSEEDN:
                region.append(int(dummies[di]))
                di += 1

            order_desc = np.argsort(-st["wb"], kind="stable")
            plist = st["act"][order_desc]
            pbuck = st["wb"][order_desc]
            placed = {i: [] for i in range(ncl)}
            ptr = 0
            for i in range(ncl - 1, -1, -1):
                cnt = int(counts[i])
                while cnt > 0 and ptr < len(plist):
                    assert pbuck[ptr] <= i
                    placed[i].append(int(plist[ptr]))
                    ptr += 1
                    cnt -= 1
            assert ptr == len(plist)
            node_order = list(region)
            for i in cls_idx:
                needi = int(counts[i]) - len(placed[i])
                fill = [int(dummies[di + j]) for j in range(needi)]
                di += needi
                node_order.extend(placed[i] + fill)
            zlist = list(st["zk"])
            while len(zlist) < Z:
                zlist.append(int(dummies[di]))
                di += 1
            node_order.extend(zlist)
            while len(node_order) < Cp:
                node_order.append(int(dummies[di]))
                di += 1
            node_order = np.array(node_order, dtype=np.int64)
            assert len(node_order) == Cp

            # slot stream
            idxh = np.full(Xp, ones_pad, dtype=np.int64)
            relc = np.full(Xp, ZERO_ENTRY, dtype=np.int64)
            rank_of = np.full(N, -1, dtype=np.int64)
            rank_of[node_order[:A]] = np.arange(A)
            lo = np.searchsorted(dst_s, n0)
            hi = np.searchsorted(dst_s, n0 + npc)
            ee = e_by_dst[lo:hi]
            ee = ee[e_act[ee]]
            if len(ee):
                rk = rank_of[dst[ee]]
                assert (rk >= 0).all()
                o2 = np.argsort(rk, kind="stable")
                ee = ee[o2]
                rks = rk[o2]
                grp = np.searchsorted(rks, np.arange(A))
                within = np.arange(len(ee)) - grp[rks]
                assert (within < w_seq[rks]).all()
                slotpos = rank_base[rks] + within * rank_stride[rks]
                if l == 0:
                    spos = np.array(
                        [SLABTAIL + pat_row_of[int(s)] for s in src[ee]],
                        dtype=np.int64)
                elif src_is_slab:
                    pc = prev_col[src[ee]]
                    assert (pc >= 0).all()
                    spos = pc
                else:
                    pc = prev_col[src[ee]]
                    assert (pc >= 0).all()
                    spos = (agoff[l] + prev_colrow0[pc]
                            + (src[ee] // NPC0) * prev_colmul[pc])
                idxh[slotpos] = spos
                relc[slotpos] = rel[ee]

            corr = np.zeros((2, 128, NSEEDN), dtype=np.float64)
            for irank in range(NSEEDN):
                n_ = int(node_order[irank])
                if not (is_seed[n_] and US[l][n_]):
                    continue
                p = pat[n_]
                kk = int(k_glob[n_])
                padpos = rank_base[irank] + \
                    np.arange(kk, wseed) * rank_stride[irank]
                idxh[padpos] = ones_pad
                relc[padpos] = bnd_entry_of[n_]
                npads = wseed - kk
                assert npads >= 1
                corr[0, :, irank] = (npads - 1) * p
                corr[1, :, irank] = (npads - 1) * p * p

            # hp indices
            idxp = np.full(Cp, SLABTAIL + ONES_ROW, dtype=np.int64)
            for j, n_ in enumerate(node_order):
                n_ = int(n_)
                if not US[l][n_]:
                    continue
                if l == 0:
                    idxp[j] = SLABTAIL + (pat_row_of[n_] if is_seed[n_]
                                          else ZERO_ROW)
                elif prev_col[n_] >= 0:
                    idxp[j] = prev_col[n_]
                else:
                    assert not is_seed[n_]
                    gi = gidx_of[n_]
                    assert gi >= 0
                    row = SROWS + len(percore[c]["gext_rows"])
                    percore[c]["gext_rows"].append(
                        gsnap[l][gi].reshape(NQ * D))
                    idxp[j] = SLABTAIL + row

            hv = np.ones((4, Cp), dtype=np.float64)
            usm = US[l][node_order]
            hv[0, usm] = rcnt[node_order[usm]]
            hv[1, usm] = rcnt[node_order[usm]]
            hv[2, usm] = scale[node_order[usm]]
            hv[3, usm] = iscale[node_order[usm]]

            new_col[node_order[usm]] = np.nonzero(usm)[0]

            percore[c]["idxh"].append(idxh)
            percore[c]["relc"].append(relc)
            percore[c]["idxp"].append(idxp)
            percore[c]["helpers"].append(hv)
            percore[c]["corr"].append(corr)
            percore[c]["colmap"].append(
                dict((int(n2), int(j2)) for j2, n2 in enumerate(node_order)
                     if US[l][n2]))

        prev_col = new_col
        prev_Cp = Cp
        prev_colrow0 = colrow0
        prev_colmul = colmul
        if l < L - 1:
            if repl:
                agoff[l + 1] = -1
            else:
                agoff[l + 1] = nxt
                nxt += NCORES * Cp
    TROWS = nxt
    assert TROWS <= 32767, TROWS

    return dict(layers=layers, percore=percore, query=query, seeds=seeds,
                pat=pat, NRELE=NRELE, bnd_entry_of=bnd_entry_of,
                GE=GE, TROWS=TROWS, t_index=t_index, US=US,
                gsnap=gsnap, gidx_of=gidx_of, is_seed=is_seed)


def build_weights(inputs, S):
    rel_W = np.asarray(inputs["rel_W"], np.float64)
    rel_b = np.asarray(inputs["rel_b"], np.float64)
    lin_W = np.asarray(inputs["lin_W"], np.float64)
    lin_b = np.asarray(inputs["lin_b"], np.float64)
    query = S["query"]
    NRELE = S["NRELE"]

    relpal = np.zeros((L, NRELE, 128), dtype=np.float32)
    for l in range(L):
        remb = (query @ rel_W[l] + rel_b[l]).reshape(B, 2 * R, D)
        relpal[l, :2 * R] = remb.transpose(1, 0, 2).reshape(2 * R, NQ * D)
        for n_old, eid in S["bnd_entry_of"].items():
            relpal[l, eid] = S["pat"][n_old]

    wbd = np.zeros((L, 13, 128, 128), dtype=bf16)
    bias = np.zeros((L, 128, 1), dtype=np.float32)
    for l in range(L):
        Wh = lin_W[l][:D]
        Wu = lin_W[l][D:].reshape(D, 4, 3, D)
        for g in range(4):
            for ks in range(3):
                blk = Wu[:, g, ks, :]
                m = np.zeros((128, 128))
                for q in range(NQ):
                    m[q * D:(q + 1) * D, q * D:(q + 1) * D] = blk
                wbd[l, g * 3 + ks] = m.astype(bf16)
        m = np.zeros((128, 128))
        for q in range(NQ):
            m[q * D:(q + 1) * D, q * D:(q + 1) * D] = Wh
        wbd[l, 12] = m.astype(bf16)
        for q in range(NQ):
            bias[l, q * D:(q + 1) * D, 0] = lin_b[l]

    return dict(relpal=relpal, wbd=wbd, bias=bias,
                mlp=(np.asarray(inputs["mlp_W1"], np.float64),
                     np.asarray(inputs["mlp_b1"], np.float64),
                     np.asarray(inputs["mlp_W2"], np.float64),
                     np.asarray(inputs["mlp_b2"], np.float64)))


def build_program(S):
    import concourse.tile as tile
    from concourse import bacc, mybir
    import contextlib

    layers = S["layers"]
    XT = sum(sl["Xp"] for sl in layers)
    CT = sum(sl["Cp"] for sl in layers)
    Cpmax = max(sl["Cp"] for sl in layers)
    Xpmax = max(sl["Xp"] for sl in layers)
    XpCmax = max(ch["Xp"] for sl in layers for ch in sl["chunks"])
    TROWS = S["TROWS"]
    GE = S["GE"]
    CpL = layers[L - 1]["Cp"]
    SR = SROWS + GE

    nc = bacc.Bacc("TRN2", target_bir_lowering=False, debug=False,
                   num_devices=NCORES)
    dtb = mybir.dt.bfloat16
    dtf = mybir.dt.float32
    dti = mybir.dt.int16
    OP = mybir.AluOpType
    AF = mybir.ActivationFunctionType

    idxh_d = nc.dram_tensor("idxh", [128, XT // 16], dti,
                            kind="ExternalInput")
    idxp_d = nc.dram_tensor("idxp", [128, CT // 16], dti,
                            kind="ExternalInput")
    rs_d = nc.dram_tensor("rs", [128, XT], dtb, kind="ExternalInput")
    help_d = nc.dram_tensor("helpers", [128, 4 * CT], dtb,
                            kind="ExternalInput")
    corr_d = nc.dram_tensor("corr", [128, L * 2 * NSEEDN], dtb,
                            kind="ExternalInput")
    wbd_d = nc.dram_tensor("wbd", [128, L * 13 * 128], dtb,
                           kind="ExternalInput")
    bias_d = nc.dram_tensor("biasl", [128, L], dtf, kind="ExternalInput")
    tstat_d = nc.dram_tensor("tstat", [SR, 128], dtb, kind="ExternalInput")
    ident_d = nc.dram_tensor("ident", [128, 128], dtb, kind="ExternalInput")
    outh_d = nc.dram_tensor("outh", [128, CpL], dtb, kind="ExternalOutput")

    tbl = nc.dram_tensor("tblhbm", [TROWS, 128], dtb, addr_space="Shared")
    slabs = [nc.dram_tensor(f"slab{i}", [SLABTAIL + SR, 128], dtb)
             for i in range(2)]

    coffs = [0]
    xoffs = [0]
    for sl in layers:
        coffs.append(coffs[-1] + sl["Cp"])
        xoffs.append(xoffs[-1] + sl["Xp"])

    with tile.TileContext(nc) as tc:
        ctx = contextlib.ExitStack()
        with ctx, nc.allow_low_precision(reason="bf16 stats by design"):
            pw = ctx.enter_context(tc.tile_pool(name="pw", bufs=1))
            pgq = ctx.enter_context(tc.tile_pool(name="pgq", bufs=6))
            pm = ctx.enter_context(tc.tile_pool(name="pm", bufs=2))
            plvl = ctx.enter_context(tc.tile_pool(name="plvl", bufs=2))
            pgrid = ctx.enter_context(tc.tile_pool(name="pgrid", bufs=3))
            phid = ctx.enter_context(tc.tile_pool(name="phid", bufs=2))
            phn = ctx.enter_context(tc.tile_pool(name="phn", bufs=3))
            pt = ctx.enter_context(tc.tile_pool(name="pt", bufs=2))
            pstg = ctx.enter_context(tc.tile_pool(name="pstg", bufs=2))
            ppsum = ctx.enter_context(tc.tile_pool(name="ppsum", bufs=2,
                                                   space="PSUM"))
            ppsT = ctx.enter_context(tc.tile_pool(name="ppsT", bufs=2,
                                                  space="PSUM"))

            for sb in slabs:
                nc.sync.dma_start(out=sb[SLABTAIL:SLABTAIL + SR],
                                  in_=tstat_d[:])
            nc.sync.dma_start(out=tbl[0:SROWS], in_=tstat_d[0:SROWS])
            ihx = pw.tile([128, XT // 16], dti, tag="ihx")
            ipx = pw.tile([128, CT // 16], dti, tag="ipx")
            rsx = pw.tile([128, XT], dtb, tag="rsx")
            hlpx = pw.tile([128, 4 * CT], dtb, tag="hlpx")
            for l in range(L):
                x0, x1 = xoffs[l], xoffs[l + 1]
                c0, c1 = coffs[l], coffs[l + 1]
                nc.sync.dma_start(out=ihx[:, x0 // 16:x1 // 16],
                                  in_=idxh_d[:, x0 // 16:x1 // 16])
                nc.sync.dma_start(out=ipx[:, c0 // 16:c1 // 16],
                                  in_=idxp_d[:, c0 // 16:c1 // 16])
                nc.sync.dma_start(out=rsx[:, x0:x1], in_=rs_d[:, x0:x1])
                nc.sync.dma_start(out=hlpx[:, 4 * c0:4 * c1],
                                  in_=help_d[:, 4 * c0:4 * c1])
            ident = pw.tile([128, 128], dtb, tag="ident")
            nc.sync.dma_start(out=ident[:], in_=ident_d[:])
            wbdx = pw.tile([128, L * 13, 128], dtb, tag="wbdx")
            nc.sync.dma_start(
                out=wbdx[:],
                in_=wbd_d[:].rearrange("p (k f) -> p k f", f=128))
            biasx = pw.tile([128, L], dtf, tag="biasx")
            nc.sync.dma_start(out=biasx[:], in_=bias_d[:])
            corx = pw.tile([128, L * 2 * NSEEDN], dtb, tag="corx")
            nc.sync.dma_start(out=corx[:], in_=corr_d[:])

            def hp_gather(l):
                Cp_l = layers[l]["Cp"]
                c0 = coffs[l]
                hpb = phid.tile([128, Cpmax], dtb, tag="hp", name="hpb")
                nc.gpsimd.dma_gather(
                    out_ap=hpb[:, :Cp_l].rearrange("p (c n) -> p c n", c=1),
                    in_ap=slabs[(l + 1) % 2][:],
                    idxs_ap=ipx[:, c0 // 16:(c0 + Cp_l) // 16],
                    num_idxs=Cp_l, num_idxs_reg=Cp_l,
                    elem_size=128, transpose=True, single_packet=False)
                return hpb[:, :Cp_l]

            hp_next = hp_gather(0)

            for l in range(L):
                sl = layers[l]
                Cp, A = sl["Cp"], sl["A"]
                xoff, coff = xoffs[l], coffs[l]
                wbd = wbdx[:, l * 13:(l + 1) * 13, :]
                biasv = biasx[:, l:l + 1]
                corrt = corx[:, l * 2 * NSEEDN:(l + 1) * 2 * NSEEDN]\
                    .rearrange("p (k f) -> p k f", k=2)
                hlp = hlpx[:, 4 * coff:4 * coff + 4 * Cp]\
                    .rearrange("p (k f) -> p k f", k=4)
                hp = hp_next
                nchunks = len(sl["chunks"])
                agl = layers[l + 1]["agoff"] if l < L - 1 else 0
                if l < L - 1 and sl["repl"]:
                    agl = 0

                src_slab = (l == 0) or layers[l - 1]["repl"]
                gsrc = slabs[(l + 1) % 2] if src_slab else tbl
                gqs = []
                for ch in sl["chunks"]:
                    Xpk = ch["Xp"]
                    sb = xoff + ch["sbase"]
                    gq = pgq.tile([128, 2, XpCmax], dtb, tag="gq",
                                  name="gq")
                    nc.gpsimd.dma_gather(
                        out_ap=gq[:, 0:1, :Xpk],
                        in_ap=gsrc[:],
                        idxs_ap=ihx[:, sb // 16:(sb + Xpk) // 16],
                        num_idxs=Xpk, num_idxs_reg=Xpk,
                        elem_size=128, transpose=True, single_packet=False)
                    gqs.append(gq)

                for kc, ch in enumerate(sl["chunks"]):
                    c0, ncols, Xpk = ch["c0"], ch["ncols"], ch["Xp"]
                    rel_a = max(A - c0, 0)
                    nce = min(ncols, _rup(max(rel_a, 1), 32))
                    sb = xoff + ch["sbase"]
                    gq = gqs[kc]
                    nc.vector.tensor_tensor(out=gq[:, 0, :Xpk],
                                            in0=gq[:, 0, :Xpk],
                                            in1=rsx[:, sb:sb + Xpk],
                                            op=OP.mult)
                    nc.scalar.activation(gq[:, 1, :Xpk], gq[:, 0, :Xpk],
                                         AF.Square)

                    # grid planes: 0=sum 1=sq 2=max 3=min (chunk-local)
                    gr = pgrid.tile([128, 4, 512], dtb, tag="grid",
                                    name="gr")
                    if rel_a < nce:
                        nc.vector.memset(gr[:, :, rel_a:nce], 0.0)
                    min_eng = None
                    for (toff, tnode, nseg, w) in ch["runs"]:
                        _tree(nc, plvl, OP.add, gq, 0, 2, toff, nseg, w,
                              gr, 0, tnode, mybir, XpCmax)
                        _tree(nc, plvl, OP.max, gq, 0, 1, toff, nseg, w,
                              gr, 2, tnode, mybir, XpCmax)
                        _tree(nc, plvl, OP.min, gq, 0, 1, toff, nseg, w,
                              gr, 3, tnode, mybir, XpCmax,
                              eng=min_eng)

                    if kc == 0:
                        nc.vector.tensor_tensor(out=gr[:, 0:2, :NSEEDN],
                                                in0=gr[:, 0:2, :NSEEDN],
                                                in1=corrt[:],
                                                op=OP.subtract)
                        mm0 = NSEEDN
                    else:
                        mm0 = 0
                    nc.vector.tensor_scalar_max(gr[:, 2, mm0:nce],
                                                gr[:, 2, mm0:nce], 0.0)
                    nc.vector.tensor_scalar_min(gr[:, 3, mm0:nce],
                                                gr[:, 3, mm0:nce], 0.0)
                    nc.vector.tensor_tensor(out=gr[:, 0:2, :nce],
                                            in0=gr[:, 0:2, :nce],
                                            in1=hlp[:, 0:2, c0:c0 + nce],
                                            op=OP.mult)
                    msc = pm.tile([128, 512], dtb, tag="msc",
                                  name="msc")[:, :nce]
                    nc.scalar.activation(msc[:], gr[:, 0, :nce],
                                         AF.Square)
                    nc.vector.tensor_tensor(out=gr[:, 1, :nce],
                                            in0=gr[:, 1, :nce],
                                            in1=msc[:], op=OP.subtract)
                    nc.vector.tensor_scalar_max(gr[:, 1, :nce],
                                                gr[:, 1, :nce], EPS)
                    nc.scalar.activation(gr[:, 1, :nce],
                                         gr[:, 1, :nce], AF.Sqrt)

                    hnew = phn.tile([128, 512], dtb, tag="hn",
                                    name="hnew")[:, :ncols]
                    gl = [gr[:, 0], gr[:, 2], gr[:, 3], gr[:, 1]]
                    ps = [ppsum.tile([128, 512], dtf, tag=f"ps{k2}",
                                     name=f"ps{k2}") for k2 in range(3)]
                    for ks in range(3):
                        for g in range(4):
                            nc.tensor.matmul(
                                ps[ks][:, :nce], wbd[:, g * 3 + ks, :],
                                gl[g][:, :nce],
                                start=(g == 0),
                                stop=(g == 3 and ks != 0))
                    nc.tensor.matmul(ps[0][:, :nce], wbd[:, 12, :],
                                     hp[:, c0:c0 + nce],
                                     start=False, stop=True)
                    t1 = pt.tile([128, 512], dtb, tag="t1")
                    nc.vector.tensor_tensor(out=t1[:, :nce],
                                            in0=ps[1][:, :nce],
                                            in1=hlp[:, 2, c0:c0 + nce],
                                            op=OP.mult)
                    t2 = pt.tile([128, 512], dtb, tag="t2")
                    nc.vector.tensor_tensor(out=t2[:, :nce],
                                            in0=ps[2][:, :nce],
                                            in1=hlp[:, 3, c0:c0 + nce],
                                            op=OP.mult)
                    nc.vector.tensor_tensor(out=t1[:, :nce],
                                            in0=t1[:, :nce],
                                            in1=t2[:, :nce], op=OP.add)
                    nc.vector.tensor_tensor(out=t1[:, :nce],
                                            in0=t1[:, :nce],
                                            in1=ps[0][:, :nce],
                                            op=OP.add)
                    nc.scalar.activation(hnew[:, :nce], t1[:, :nce],
                                         AF.Relu, bias=biasv)
                    if nce < ncols and l < L - 1:
                        nc.vector.memset(hnew[:, nce:ncols], 0.0)

                    if l == L - 1:
                        nc.sync.dma_start(out=outh_d[:, c0:c0 + nce],
                                          in_=hnew[:, :nce])
                    else:
                        slab = slabs[l % 2]
                        for rk in range(ncols // 128):
                            psT = ppsT.tile([128, 128], dtb, tag="psT")
                            nc.tensor.transpose(
                                psT[:],
                                hnew[:, rk * 128:(rk + 1) * 128],
                                ident[:])
                            stg = pstg.tile([128, 128], dtb, tag="stg")
                            nc.scalar.activation(stg[:], psT[:], AF.Copy)
                            nc.sync.dma_start(
                                out=slab[c0 + rk * 128:
                                         c0 + (rk + 1) * 128, :],
                                in_=stg[:])
                        if kc == nchunks - 1:
                            hp_next = hp_gather(l + 1)
                        if not sl["repl"]:
                            nc.gpsimd.collective_compute(
                                "AllGather", OP.bypass,
                                replica_groups=[list(range(NCORES))],
                                ins=[slab[c0:c0 + ncols]],
                                outs=[tbl[agl + ch["rowbase"]:
                                          agl + ch["rowbase"]
                                          + NCORES * ncols]
                                      .rearrange("(c n) d -> c n d",
                                                 c=NCORES)])

    nc.compile()
    return nc


def _tree(nc, plvl, op, src3, p0, np_, toff, nseg, w, grid3, g0, noff,
          mybir, lvlw, eng=None):
    if eng is None:
        eng = nc.vector
    dtb = mybir.dt.bfloat16
    gout = grid3[:, g0:g0 + np_, noff:noff + nseg]
    if w == 1:
        eng.tensor_copy(gout, src3[:, p0:p0 + np_, toff:toff + nseg])
        return
    cur, cp0, cof, m = src3, p0, toff, w
    while m > 1:
        h = m // 2
        odd = m - 2 * h
        last = (h == 1 and odd == 0)
        if last:
            nxt, nof = grid3[:, g0:g0 + np_], noff
        else:
            nxt = plvl.tile([128, 2, lvlw // 2], dtb, tag="lvl",
                            name="lvl")[:, :np_]
            nof = 0
        eng.tensor_tensor(
            out=nxt[:, :, nof:nof + h * nseg],
            in0=cur[:, cp0:cp0 + np_, cof:cof + h * nseg],
            in1=cur[:, cp0:cp0 + np_, cof + h * nseg:cof + 2 * h * nseg],
            op=op)
        if odd:
            if h == 1:
                eng.tensor_tensor(
                    out=gout,
                    in0=nxt[:, :, nof:nof + nseg],
                    in1=cur[:, cp0:cp0 + np_,
                            cof + 2 * h * nseg:cof + (2 * h + 1) * nseg],
                    op=op)
                return
            eng.tensor_tensor(
                out=nxt[:, :, nof:nof + nseg],
                in0=nxt[:, :, nof:nof + nseg],
                in1=cur[:, cp0:cp0 + np_,
                        cof + 2 * h * nseg:cof + (2 * h + 1) * nseg],
                op=op)
        cur, cp0, cof, m = nxt, 0, nof, h


_RUN_STATE = {}


def kernel(**inputs):
    from concourse.bass_utils import run_bass_kernel_spmd

    S = build_host(inputs)
    W = build_weights(inputs, S)
    nc = build_program(S)

    layers = S["layers"]
    XT = sum(sl["Xp"] for sl in layers)
    CT = sum(sl["Cp"] for sl in layers)
    GE = S["GE"]

    # static table rows (shared): zero, ones, patterns
    tstat_base = np.zeros((SROWS + GE, 128), dtype=bf16)
    tstat_base[ONES_ROW] = bf16(1.0)
    for n_old, p in S["pat"].items():
        tstat_base[PAT_BASE + (S["bnd_entry_of"][n_old] - ZERO_ENTRY - 1)] \
            = p.astype(bf16)

    in_maps = []
    for c in range(NCORES):
        pc = S["percore"][c]
        ihw = _wrap_idx(np.concatenate(pc["idxh"]))
        ipw = _wrap_idx(np.concatenate(pc["idxp"]))
        relc = np.concatenate(pc["relc"])
        rsb = np.zeros((128, XT), dtype=bf16)
        xo = 0
        for l in range(L):
            nl = len(pc["relc"][l])
            rsb[:, xo:xo + nl] = \
                W["relpal"][l][relc[xo:xo + nl]].T.astype(bf16)
            xo += nl
        hlpflat = np.concatenate(
            [pc["helpers"][l].reshape(-1) for l in range(L)])
        hlpb = np.broadcast_to(hlpflat[None, :].astype(bf16),
                               (128, 4 * CT)).copy()
        corrflat = np.concatenate(
            [pc["corr"][l].transpose(1, 0, 2).reshape(128, 2 * NSEEDN)
             for l in range(L)], axis=1).astype(bf16)
        tstat = tstat_base.copy()
        assert len(pc["gext_rows"]) <= GE
        for i, row in enumerate(pc["gext_rows"]):
            tstat[SROWS + i] = row.astype(bf16)
        in_maps.append(dict(
            idxh=ihw, idxp=ipw, rs=rsb, helpers=hlpb, corr=corrflat,
            wbd=np.ascontiguousarray(
                W["wbd"].transpose(2, 0, 1, 3).reshape(128, -1)),
            biasl=np.ascontiguousarray(W["bias"][:, :, 0].T),
            tstat=tstat, ident=np.eye(128, dtype=bf16)))

    res = run_bass_kernel_spmd(nc, in_maps, core_ids=list(range(NCORES)),
                               trace=bool(os.environ.get("NBF_TRACE")))
    _RUN_STATE["exec_time_ns"] = res.exec_time_ns

    t_index = S["t_index"]
    US5 = S["US"][L - 1]
    outs = []
    for c in range(NCORES):
        o = np.asarray(res.results[c]["outh"])
        if o.dtype != bf16:
            o = o.view(bf16)
        outs.append(o.astype(np.float64))
    gL = S["gsnap"][L]
    gidx_of = S["gidx_of"]
    hidvec = {}
    for t in np.unique(t_index):
        t = int(t)
        if US5[t]:
            c = t // NPC0
            col = S["percore"][c]["colmap"][L - 1][t]
            hidvec[t] = outs[c][:, col]
        else:
            gi = gidx_of[t]
            assert gi >= 0
            hidvec[t] = gL[gi].reshape(NQ * D)

    mlp_W1, mlp_b1, mlp_W2, mlp_b2 = W["mlp"]
    query = S["query"]
    Kk = t_index.shape[1]
    score = np.zeros((B, Kk), dtype=np.float32)
    for q in range(B):
        feat = np.stack([
            np.concatenate([hidvec[int(t)][q * D:(q + 1) * D], query[q]])
            for t in t_index[q]])
        hdd = np.maximum(feat @ mlp_W1 + mlp_b1, 0)
        score[q] = ((hdd @ mlp_W2 + mlp_b2)[:, 0]).astype(np.float32)
    return score
